# revision 14
# baseline (speedup 1.0000x reference)
"""Mat2Twist Trainium2 kernel: batch of 3x3 rotation matrices -> twist vectors.

For each matrix R:  tr = trace(R); x = (tr-1)/2 = cos(theta)
  theta = arccos(x) = pi/2 - arctan(x / sqrt(1 - x^2))
  w = [R21-R12, R02-R20, R10-R01]   (unnormalized axis, |w| = 2 sin theta)
  out = theta/(2 sin theta) * w

Data-parallel over 8 NeuronCores. The host pre-arranges each core's shard
tile-major/component-major (PERM) so every on-chip op and DMA is unit-stride.
Per chunk (m matrices per partition), tile X = [minu(3m) | subt(3m) | R00 |
R11 | R22]:
  w    = X[0:3m] - X[3m:6m]       in place into X[0:3m]   [DVE]
  tr   = R00 + R11 + R22                                   [GpSimd TT x2]
  s2x  = tr - 1                   = 2x                     [DVE tensor_scalar 2x]
  v    = Square(0.5*s2x)          = x^2                    [ACT]
  lg   = Ln(1 - v)                                         [ACT]
  r    = Exp(-0.5*lg)             = 1/sin theta            [ACT]
  xr   = (0.5*s2x) * r            = cot theta              [DVE STT]
  t    = Arctan(xr)               = pi/2 - theta           [ACT, in place]
  msc2 = (t - pi/2) * r           = -theta/sin theta       [DVE STT]
  out_k = (-0.5*msc2) * w_k       = theta/(2 sin th)*w_k   [DVE STT x3, in place]
  out DMA from X[0:3m]            (ACT-HWDGE ring; inputs ride the SP ring)

Chunks are processed in PAIRS (stage1(i), stage1(i+1), stage2(i),
stage2(i+1)) so the ACT engine switches function tables (ln/exp set <->
arctan set) once per pair instead of twice per chunk; table loads cost
~1.3 us each.
"""

import numpy as np

import concourse.bass as bass
import concourse.mybir as mybir
from concourse.tile import TileContext
from concourse.bass_utils import run_bass_kernel_spmd

B = 4194304
NCORES = 8
P = 128
N_C = B // NCORES        # 524288 matrices per core
MPP = N_C // P           # 4096 matrices per partition
MS = [512, 1024, 1024, 1024, 512]   # per-chunk matrices per partition
assert sum(MS) == MPP

# component order in DRAM (flat 3x3 index): minuends, subtrahends, diagonal
PERM = [7, 2, 3, 5, 6, 1, 0, 4, 8]

F32 = mybir.dt.float32
ACT = mybir.ActivationFunctionType
ALU = mybir.AluOpType
PI_2 = float(np.pi / 2.0)
MAXM = max(MS)


def _split_multi_waits(nc):
    """This container's walrus build rejects >1 sem-wait per instruction
    ("Too many sync wait commands"); hoist extras onto preceding NOPs."""
    for f in nc.m.functions:
        for blk in f.blocks:
            il = blk.instructions
            new = []
            for ins in il:
                si = ins.sync_info
                if si is not None and si.on_wait is not None and len(si.on_wait) > 1:
                    waits = list(si.on_wait)
                    for j, w in enumerate(waits[:-1]):
                        nop = mybir.InstNoOp(name=f"{ins.name}-ws{j}", engine=ins.engine)
                        nop.sync_info = mybir.SyncInfo(on_wait=[w], on_update=[])
                        new.append(nop)
                    ins.sync_info = mybir.SyncInfo(
                        on_wait=[waits[-1]], on_update=list(si.on_update or [])
                    )
                new.append(ins)
            il[:] = new


def _build_kernel():
    nc = bass.Bass()
    x_in = nc.dram_tensor("mat_in", [N_C * 9], F32, kind="ExternalInput")
    y_out = nc.dram_tensor("twist_out", [N_C * 3], F32, kind="ExternalOutput")

    with TileContext(nc) as tc:
        with tc.tile_pool(name="xp", bufs=4) as xp, \
             tc.tile_pool(name="tmp", bufs=2) as tmp:

            def stage1(ci, off, m):
                base = off * P * 9
                X = xp.tile([P, 9 * MAXM], F32, tag="X", name=f"X{ci}")[:, : 9 * m]
                nc.sync.dma_start(
                    out=X,
                    in_=x_in[base : base + P * 9 * m].rearrange("(p n) -> p n", p=P),
                )
                # w into X[0:3m]
                nc.vector.tensor_sub(
                    out=X[:, 0 : 3 * m], in0=X[:, 0 : 3 * m], in1=X[:, 3 * m : 6 * m]
                )
                # trace on the (otherwise idle) GpSimd engine; -1 via fast
                # 2x-mode tensor_scalar on DVE
                s2x = tmp.tile([P, MAXM], F32, tag="s2x", name=f"s2x{ci}")[:, :m]
                nc.gpsimd.tensor_add(
                    out=s2x, in0=X[:, 6 * m : 7 * m], in1=X[:, 7 * m : 8 * m]
                )
                nc.gpsimd.tensor_add(out=s2x, in0=s2x, in1=X[:, 8 * m : 9 * m])
                nc.vector.tensor_scalar(
                    out=s2x, in0=s2x, scalar1=1.0, scalar2=None, op0=ALU.subtract
                )

                v = tmp.tile([P, MAXM], F32, tag="v", name=f"v{ci}")[:, :m]
                nc.scalar.activation(v, s2x, ACT.Square, scale=0.5)
                lg = tmp.tile([P, MAXM], F32, tag="lg", name=f"lg{ci}")[:, :m]
                nc.scalar.activation(lg, v, ACT.Ln, bias=1.0, scale=-1.0)
                r = tmp.tile([P, MAXM], F32, tag="r", name=f"r{ci}")[:, :m]
                nc.scalar.activation(r, lg, ACT.Exp, scale=-0.5)
                return X, s2x, r

            def stage2(ci, off, m, X, s2x, r):
                xr = tmp.tile([P, MAXM], F32, tag="xr", name=f"xr{ci}")[:, :m]
                nc.vector.scalar_tensor_tensor(
                    out=xr, in0=s2x, scalar=0.5, in1=r,
                    op0=ALU.mult, op1=ALU.mult,
                )
                nc.scalar.activation(xr, xr, ACT.Arctan)
                msc2 = tmp.tile([P, MAXM], F32, tag="msc2", name=f"msc2{ci}")[:, :m]
                nc.vector.scalar_tensor_tensor(
                    out=msc2, in0=xr, scalar=PI_2, in1=r,
                    op0=ALU.subtract, op1=ALU.mult,
                )
                for k in range(3):
                    blk = X[:, k * m : (k + 1) * m]
                    nc.vector.scalar_tensor_tensor(
                        out=blk, in0=msc2, scalar=-0.5, in1=blk,
                        op0=ALU.mult, op1=ALU.mult,
                    )
                dst = y_out[off * P * 3 : (off + m) * P * 3].rearrange(
                    "(p n) -> p n", p=P
                )
                nc.scalar.dma_start(out=dst, in_=X[:, 0 : 3 * m])

            offs = [0] + list(np.cumsum(MS)[:-1].astype(int))
            # pair chunks: stage1(i), stage1(i+1), stage2(i), stage2(i+1)
            # so ACT loads the arctan table once per pair.
            pend = []
            for cj in range(len(MS)):
                pend.append((cj, offs[cj], MS[cj], *stage1(cj, offs[cj], MS[cj])))
                if len(pend) == 2:
                    for args in pend:
                        stage2(*args)
                    pend = []
            for args in pend:
                stage2(*args)

    _split_multi_waits(nc)
    return nc


_NC_CACHE = []


def _host_pack(mat_batch: np.ndarray) -> np.ndarray:
    """[B,3,3] -> [NCORES, N_C*9] tile-major/component-major PERM layout."""
    flat = np.ascontiguousarray(mat_batch, dtype=np.float32).reshape(
        NCORES, N_C, 9
    )
    out = np.empty((NCORES, N_C * 9), np.float32)
    pos = 0
    for m, off in zip(MS, np.concatenate([[0], np.cumsum(MS)[:-1]])):
        off = int(off)
        chunk = flat[:, off * P : (off + m) * P, :].reshape(NCORES, P, m, 9)
        sz = P * m * 9
        out[:, pos : pos + sz] = (
            chunk.transpose(0, 1, 3, 2)[:, :, PERM, :].reshape(NCORES, sz)
        )
        pos += sz
    return out


def _host_unpack(res_list) -> np.ndarray:
    out = np.empty((B, 3), np.float32)
    o = out.reshape(NCORES, N_C, 3)
    for i, r in enumerate(res_list):
        y = r["twist_out"]
        pos = 0
        for m, off in zip(MS, np.concatenate([[0], np.cumsum(MS)[:-1]])):
            off = int(off)
            sz = P * m * 3
            blk = y[pos : pos + sz].reshape(P, 3, m)
            o[i, off * P : (off + m) * P, :] = blk.transpose(0, 2, 1).reshape(
                P * m, 3
            )
            pos += sz
    return out


def kernel(mat_batch: np.ndarray) -> np.ndarray:
    if not _NC_CACHE:
        _NC_CACHE.append(_build_kernel())
    nc = _NC_CACHE[0]

    packed = _host_pack(mat_batch)
    in_maps = [{"mat_in": packed[i]} for i in range(NCORES)]
    res = run_bass_kernel_spmd(nc, in_maps, core_ids=list(range(NCORES)))
    return _host_unpack(res.results)


# revision 15
# speedup vs baseline: 1.0670x; 1.0670x over previous
"""Mat2Twist Trainium2 kernel: batch of 3x3 rotation matrices -> twist vectors.

For each matrix R:  tr = trace(R); x = (tr-1)/2 = cos(theta)
  theta = arccos(x) = pi/2 - arctan(x / sqrt(1 - x^2))
  w = [R21-R12, R02-R20, R10-R01]   (unnormalized axis, |w| = 2 sin theta)
  out = theta/(2 sin theta) * w

Data-parallel over 8 NeuronCores. The host pre-arranges each core's shard
tile-major/component-major (PERM) so every on-chip op and DMA is unit-stride.
Per chunk (m matrices per partition), tile X = [minu(3m)|subt(3m)|R00|R11|R22]:
  tr   = R00 + R11 + R22                           [GpSimd TT x2]
  v    = Square(0.5*tr - 0.5)     = x^2            [ACT]
  lg   = Ln(1 - v)                                 [ACT]
  r    = Exp(-0.5*lg)             = 1/sin theta    [ACT]
  w    = X[0:3m] - X[3m:6m]       in place         [DVE TT]
  xr   = (tr - 1) * r             = 2 cot theta    [DVE STT]
  t    = Arctan(0.5*xr)           = pi/2 - theta   [ACT, in place]
  msc2 = (t - pi/2) * r           = -theta/sin th  [DVE STT]
  out_k = (-0.5*msc2) * w_k                        [DVE STT x3, in place]

All engines are in-order queues, so the emission is SOFTWARE-PIPELINED with a
4-chunk skew: each loop iteration issues DMA(i) [SP ring], trace adds(i-1)
[GpSimd], xr(i-3) [DVE first so ACT's arctan isn't blocked], sq/ln/exp(i-2)
[ACT] + w-sub(i-2) [DVE], arctan(i-3) [ACT], and msc2/muls/out-DMA(i-4)
[DVE + ACT-HWDGE ring]. That keeps every engine's queue free of long
cross-engine waits; the chunk cadence is then input-DMA-bound.
"""

import numpy as np

import concourse.bass as bass
import concourse.mybir as mybir
from concourse.tile import TileContext
from concourse.bass_utils import run_bass_kernel_spmd

B = 4194304
NCORES = 8
P = 128
N_C = B // NCORES        # 524288 matrices per core
MPP = N_C // P           # 4096 matrices per partition
MS = [512] * 8           # per-chunk matrices per partition
assert sum(MS) == MPP

# component order in DRAM (flat 3x3 index): minuends, subtrahends, diagonal
PERM = [7, 2, 3, 5, 6, 1, 0, 4, 8]

F32 = mybir.dt.float32
ACT = mybir.ActivationFunctionType
ALU = mybir.AluOpType
PI_2 = float(np.pi / 2.0)
MAXM = max(MS)


def _split_multi_waits(nc):
    """This container's walrus build rejects >1 sem-wait per instruction
    ("Too many sync wait commands"); hoist extras onto preceding NOPs."""
    for f in nc.m.functions:
        for blk in f.blocks:
            il = blk.instructions
            new = []
            for ins in il:
                si = ins.sync_info
                if si is not None and si.on_wait is not None and len(si.on_wait) > 1:
                    waits = list(si.on_wait)
                    for j, w in enumerate(waits[:-1]):
                        nop = mybir.InstNoOp(name=f"{ins.name}-ws{j}", engine=ins.engine)
                        nop.sync_info = mybir.SyncInfo(on_wait=[w], on_update=[])
                        new.append(nop)
                    ins.sync_info = mybir.SyncInfo(
                        on_wait=[waits[-1]], on_update=list(si.on_update or [])
                    )
                new.append(ins)
            il[:] = new


def _register_const_ap(nc, value):
    """Mimic Bass.__init__'s register_const_ap for an extra f32 constant
    (memset + barrier happen before TileContext, same as the built-ins)."""
    tensor = nc.alloc_sbuf_tensor(f"const-f32-{value}", [128, 1], F32)
    nc.gpsimd.memset(tensor.ap(), value)
    nc.const_aps.aps[(F32, value)] = tensor.ap()
    nc.all_engine_barrier()


def _build_kernel():
    nc = bass.Bass()
    _register_const_ap(nc, -0.5)
    x_in = nc.dram_tensor("mat_in", [N_C * 9], F32, kind="ExternalInput")
    y_out = nc.dram_tensor("twist_out", [N_C * 3], F32, kind="ExternalOutput")

    n = len(MS)
    offs = [0] + list(np.cumsum(MS)[:-1].astype(int))

    with TileContext(nc) as tc:
        with tc.tile_pool(name="xp", bufs=5) as xp, \
             tc.tile_pool(name="tmp", bufs=4) as tmp:

            X_ = {}
            tr_ = {}
            r_ = {}
            xr_ = {}

            def s0_dma(i):
                m = MS[i]
                base = offs[i] * P * 9
                X_[i] = xp.tile([P, 9 * MAXM], F32, tag="X", name=f"X{i}")[:, : 9 * m]
                nc.sync.dma_start(
                    out=X_[i],
                    in_=x_in[base : base + P * 9 * m].rearrange("(p n) -> p n", p=P),
                )

            def s1_trace(i):
                m = MS[i]
                X = X_[i]
                tr = tmp.tile([P, MAXM], F32, tag="tr", name=f"tr{i}")[:, :m]
                nc.gpsimd.tensor_add(
                    out=tr, in0=X[:, 6 * m : 7 * m], in1=X[:, 7 * m : 8 * m]
                )
                nc.gpsimd.tensor_add(out=tr, in0=tr, in1=X[:, 8 * m : 9 * m])
                tr_[i] = tr

            def s2_scalar_chain(i):
                m = MS[i]
                X, tr = X_[i], tr_[i]
                v = tmp.tile([P, MAXM], F32, tag="v", name=f"v{i}")[:, :m]
                nc.scalar.activation(v, tr, ACT.Square, scale=0.5, bias=-0.5)
                lg = tmp.tile([P, MAXM], F32, tag="lg", name=f"lg{i}")[:, :m]
                nc.scalar.activation(lg, v, ACT.Ln, bias=1.0, scale=-1.0)
                r = tmp.tile([P, MAXM], F32, tag="r", name=f"r{i}")[:, :m]
                nc.scalar.activation(r, lg, ACT.Exp, scale=-0.5)
                r_[i] = r
                nc.vector.tensor_sub(
                    out=X[:, 0 : 3 * m], in0=X[:, 0 : 3 * m], in1=X[:, 3 * m : 6 * m]
                )

            def s3a_xr(i):
                m = MS[i]
                xr = tmp.tile([P, MAXM], F32, tag="xr", name=f"xr{i}")[:, :m]
                nc.vector.scalar_tensor_tensor(
                    out=xr, in0=tr_[i], scalar=1.0, in1=r_[i],
                    op0=ALU.subtract, op1=ALU.mult,
                )
                xr_[i] = xr

            def s3b_arctan(i):
                nc.scalar.activation(xr_[i], xr_[i], ACT.Arctan, scale=0.5)

            def s4_out(i):
                m = MS[i]
                X = X_[i]
                msc2 = tmp.tile([P, MAXM], F32, tag="msc2", name=f"msc2{i}")[:, :m]
                nc.vector.scalar_tensor_tensor(
                    out=msc2, in0=xr_[i], scalar=PI_2, in1=r_[i],
                    op0=ALU.subtract, op1=ALU.mult,
                )
                for k in range(3):
                    blk = X[:, k * m : (k + 1) * m]
                    nc.vector.scalar_tensor_tensor(
                        out=blk, in0=msc2, scalar=-0.5, in1=blk,
                        op0=ALU.mult, op1=ALU.mult,
                    )
                dst = y_out[offs[i] * P * 3 : (offs[i] + MS[i]) * P * 3].rearrange(
                    "(p n) -> p n", p=P
                )
                nc.scalar.dma_start(out=dst, in_=X[:, 0 : 3 * m])

            for i in range(n + 4):
                if i < n:
                    s0_dma(i)
                if 0 <= i - 1 < n:
                    s1_trace(i - 1)
                if 0 <= i - 3 < n:
                    s3a_xr(i - 3)
                if 0 <= i - 2 < n:
                    s2_scalar_chain(i - 2)
                if 0 <= i - 3 < n:
                    s3b_arctan(i - 3)
                if 0 <= i - 4 < n:
                    s4_out(i - 4)

    _split_multi_waits(nc)
    return nc


_NC_CACHE = []


def _host_pack(mat_batch: np.ndarray) -> np.ndarray:
    """[B,3,3] -> [NCORES, N_C*9] tile-major/component-major PERM layout."""
    flat = np.ascontiguousarray(mat_batch, dtype=np.float32).reshape(
        NCORES, N_C, 9
    )
    out = np.empty((NCORES, N_C * 9), np.float32)
    pos = 0
    for m, off in zip(MS, np.concatenate([[0], np.cumsum(MS)[:-1]])):
        off = int(off)
        chunk = flat[:, off * P : (off + m) * P, :].reshape(NCORES, P, m, 9)
        sz = P * m * 9
        out[:, pos : pos + sz] = (
            chunk.transpose(0, 1, 3, 2)[:, :, PERM, :].reshape(NCORES, sz)
        )
        pos += sz
    return out


def _host_unpack(res_list) -> np.ndarray:
    out = np.empty((B, 3), np.float32)
    o = out.reshape(NCORES, N_C, 3)
    for i, r in enumerate(res_list):
        y = r["twist_out"]
        pos = 0
        for m, off in zip(MS, np.concatenate([[0], np.cumsum(MS)[:-1]])):
            off = int(off)
            sz = P * m * 3
            blk = y[pos : pos + sz].reshape(P, 3, m)
            o[i, off * P : (off + m) * P, :] = blk.transpose(0, 2, 1).reshape(
                P * m, 3
            )
            pos += sz
    return out


def kernel(mat_batch: np.ndarray) -> np.ndarray:
    if not _NC_CACHE:
        _NC_CACHE.append(_build_kernel())
    nc = _NC_CACHE[0]

    packed = _host_pack(mat_batch)
    in_maps = [{"mat_in": packed[i]} for i in range(NCORES)]
    res = run_bass_kernel_spmd(nc, in_maps, core_ids=list(range(NCORES)))
    return _host_unpack(res.results)
